# revision 10
# baseline (speedup 1.0000x reference)
"""Binarized linear kernel for Trainium2 (8 NeuronCores).

Problem: per-direction binary "match count" GEMM.
  input        (B=64, D=128, I=512)  bool
  weight_noise (D=128, O=512, I=512) bool
  bias_noise   (D=128, O=512)        float32
  out[b,d,o] = (#matches(input[b,d,:], weight_noise[d,:,:]) > bias_noise[d,o])

Math: with +/-1 encoding x~=2x-1, w~=2w-1:
  matches = (I + sum_i x~ w~) / 2, so
  out = (dotpm > t), t = 2*bias - I rounded to the next odd integer
  (dotpm is even, so the odd-integer compare is exactly equivalent).
Host pre-encodes +/-1 in fp8_e4m3 (exact).

Threshold folding: t = 128*a + 16*b + c with a in [-4,4], b,c in [-8,8] --
all exact fp8 integers (note ml_dtypes float8_e4m3 tops out at 240, so the
largest scale must stay below that).  A K=4 "threshold chunk" per direction
contracts stationary constants (128,16,1,0) against moving rows (-a,-b,-c,0),
so PSUM
accumulates dotpm - t directly and the epilogue is a compare against
IMMEDIATE ZERO (no threshold DMA, no broadcast).  All arithmetic is exact
integer in fp32 PSUM => bit-identical to the reference.

Sharding: D across the 8 cores (16 directions each), fully independent.

v3: thresholds folded into the GEMM; weight DMA split + balanced across
both HWDGE queues (Sync + ACT); no warm-up or broadcast matmuls; per-pair
pipeline matmul -> DVE compare(psum>0) -> output slice DMA.
  Sync: x half 0, w pairs 0,2,4,6, out slices 2,3
  ACT:  x half 1, wthr, w pairs 1,3,5,7, out slices 0,1
  PE:   per pair: thr matmul (K=4) then 4 accumulating data matmuls,
        even/odd directions packed in PE column halves via tile_position
  DVE:  per pair: psum[128,512] > 0 -> uint8
  POOL: final semaphore cleanup
"""

import numpy as np

import sys

for _p in ("/opt/trn_rl_repo",):
    if _p not in sys.path:
        sys.path.insert(0, _p)

B, D, O, I = 64, 128, 512, 512
NCORES = 8
DL = D // NCORES   # directions per core (16)
NP = DL // 2       # direction pairs per core (8)
KC = I // 128      # contraction chunks of 128 (4)
NB = 8             # PSUM banks of [128, 512]: all 16KB/partition
XF = NP * KC * 2 * B   # xt data free size (4096)
XT = XF

_NC_CACHE = {}


def _build_bass():
    import concourse.mybir as mybir
    from concourse import bacc

    fp8 = mybir.dt.float8e4
    u8 = mybir.dt.uint8
    f32 = mybir.dt.float32

    nc = bacc.Bacc("TRN2")
    # DRAM layouts (host pre-arranged, DMAs fully contiguous):
    #   xt  [128, (p c j b)] : xt[k, p, c, j, b] = xs[b, d0+2p+j, c*128+k]
    #   wt  [NP, 128, (j c o)] : wt[p, k, j, c, o] = ws[d0+2p+j, o, c*128+k]
    #   wthr [8, (p o) + 128] : rows 0:3 = (-a,-b,-c) of even dir, 4:7 odd
    #        dir; trailing 128 cols = column-masked scale constants (lhsT)
    #   out [128, (p o)] u8 : rows 0:64 even dir of pair, 64:128 odd dir
    xt_d = nc.dram_tensor("xt", [128, XT], fp8, kind="ExternalInput")
    wt_d = nc.dram_tensor("wt", [NP, 128, 2 * KC * O], fp8, kind="ExternalInput")
    wthr_d = nc.dram_tensor("wthr", [8, NP * O + 128], fp8, kind="ExternalInput")
    out_d = nc.dram_tensor("out", [128, NP * O], u8, kind="ExternalOutput")

    from contextlib import ExitStack

    with ExitStack() as ctx:
        x_sb = ctx.enter_context(nc.sbuf_tensor("x_sb", [128, XT], fp8))
        w_sb = ctx.enter_context(nc.sbuf_tensor("w_sb", [128, NP * 2 * KC * O], fp8))
        wthr_sb = ctx.enter_context(nc.sbuf_tensor("wthr_sb", [8, NP * O + 128], fp8))
        out_sb = ctx.enter_context(nc.sbuf_tensor("out_sb", [128, NP * O], u8))
        psum = ctx.enter_context(nc.psum_tensor([128, NB * O], f32))
        sem_x = [ctx.enter_context(nc.semaphore(f"sem_x{k}")) for k in range(2)]
        sem_w = [ctx.enter_context(nc.semaphore(f"sem_w{k}")) for k in range(NP)]
        sem_wt = ctx.enter_context(nc.semaphore("sem_wt"))
        sem_pe = ctx.enter_context(nc.semaphore("sem_pe"))
        sem_dve = ctx.enter_context(nc.semaphore("sem_dve"))
        sem_out = ctx.enter_context(nc.semaphore("sem_out"))
        block = ctx.enter_context(nc.Block())

        xv = x_sb[:, :].rearrange("k (p c j b) -> k p c j b", p=NP, c=KC, j=2)
        x5 = wthr_sb[0:8, NP * O : NP * O + 128]  # masked scale constants
        wv = w_sb[:, :].rearrange("k (p j c o) -> k p j c o", p=NP, j=2, c=KC)
        w5 = wthr_sb[:, 0 : NP * O].rearrange("k (p o) -> k p o", p=NP)

        XH = XF // 2  # first half: pairs 0-3
        CW = 2 * KC * O  # free bytes per pair chunk
        # PE/DVE consume pairs in expected weight-arrival order: Q1 carries
        # x0,w0,w2,w4,w6; Q10 (whose descriptor generation ramps ~2.5us
        # later) carries wthr,x1,w1,w3,w5,w7.
        ORDER = (0, 2, 1, 4, 3, 6, 5, 7)

        def w_dma(eng, p):
            eng.dma_start(
                w_sb[:, p * CW : (p + 1) * CW], wt_d[p, :, :]
            ).then_inc(sem_w[p], 16)

        def out_dma(eng, k):
            # k-th completed compare (ORDER[k]); last pair in two halves
            eng.wait_ge(sem_dve, k + 1)
            if k < NP - 1:
                p = ORDER[k]
                lo, hi = p * O, (p + 1) * O
            else:
                p = ORDER[NP - 1]
                h = k - (NP - 1)
                lo = p * O + h * O // 2
                hi = p * O + (h + 1) * O // 2
            eng.dma_start(out_d[:, lo:hi], out_sb[:, lo:hi]).then_inc(sem_out, 16)

        @block.sync
        def _(sync):
            sync.dma_start(x_sb[:, 0:XH], xt_d[:, 0:XH]).then_inc(sem_x[0], 16)
            w_dma(sync, 0)
            w_dma(sync, 2)
            w_dma(sync, 4)
            w_dma(sync, 6)
            for k in (1, 3, 5, 7, 8):
                out_dma(sync, k)
            sync.wait_ge(sem_out, 144)

        @block.scalar
        def _(sc):
            sc.dma_start(wthr_sb[:, :], wthr_d[:, :]).then_inc(sem_wt, 16)
            sc.dma_start(x_sb[:, XH:], xt_d[:, XH:]).then_inc(sem_x[1], 16)
            w_dma(sc, 1)
            w_dma(sc, 3)
            w_dma(sc, 5)
            w_dma(sc, 7)
            for k in (0, 2, 4, 6):
                out_dma(sc, k)

        @block.gpsimd
        def _(g):
            # cleanup: reset sems so the NEFF can be re-executed
            g.wait_ge(sem_out, 144)
            all_sems = [*sem_x, *sem_w, sem_wt, sem_pe, sem_dve, sem_out]
            nums = sorted(s.num for s in all_sems)
            lo, hi = nums[0], nums[-1]
            assert nums == list(range(lo, hi + 1)), nums
            g.dma_reset(range(lo, hi + 1))
            g.sem_clear(range(lo, hi + 1))

        N_WARM = 12

        @block.tensor
        def _(t):
            # Throwaway matmuls during the DMA-wait window: the HAM power
            # controller caps PE at ~50% until several us of PE busy time
            # have accumulated, so burn that budget before the real work.
            # They scribble on psum bank 7, which pair 7's start=True resets.
            for _ in range(N_WARM):
                t.matmul(
                    psum[0:B, (NB - 1) * O : (NB - 1) * O + 256],
                    x_sb[:, 0:B],
                    w_sb[:, 0:256],
                    start=True,
                    stop=True,
                )
            # All 8 threshold matmuls run up front, inside the DMA-wait
            # window: they only need the (tiny, early) wthr transfer.  K=8
            # covers both directions of a pair via the column-masked
            # stationary constants: psum bank p := -t
            t.wait_ge(sem_wt, 16)
            for p in range(NP):
                t.matmul(
                    psum[:, p * O : (p + 1) * O],
                    x5,
                    w5[:, p, :],
                    start=True,
                    stop=False,
                    tile_position=(0, 0),
                )
            t.wait_ge(sem_x[0], 16)
            seen_x1 = False
            for p in ORDER:
                if p >= NP // 2 and not seen_x1:
                    t.wait_ge(sem_x[1], 16)
                    seen_x1 = True
                bank_ap = psum[:, p * O : (p + 1) * O]
                t.wait_ge(sem_w[p], 16)
                mm = None
                for c in range(KC):
                    # even direction -> array columns 0:64, psum rows 0:64
                    t.matmul(
                        bank_ap[0:B, :],
                        xv[:, p, c, 0, :],
                        wv[:, p, 0, c, :],
                        start=False,
                        stop=(c == KC - 1),
                        tile_position=(0, 0),
                    )
                    # odd direction -> array columns 64:128, psum rows 64:128
                    mm = t.matmul(
                        bank_ap[B : 2 * B, :],
                        xv[:, p, c, 1, :],
                        wv[:, p, 1, c, :],
                        start=False,
                        stop=(c == KC - 1),
                        tile_position=(0, 64),
                    )
                mm.then_inc(sem_pe, 1)

        @block.vector
        def _(v):
            for k, p in enumerate(ORDER):
                v.wait_ge(sem_pe, k + 1)
                if k < NP - 1:
                    v.tensor_scalar(
                        out=out_sb[:, p * O : (p + 1) * O],
                        in0=psum[:, p * O : (p + 1) * O],
                        scalar1=0.0,
                        scalar2=None,
                        op0=mybir.AluOpType.is_gt,
                    ).then_inc(sem_dve, 1)
                else:
                    # last pair: halves, so the final out DMA (and its DGE
                    # latency) overlaps the second half's compare
                    for h in range(2):
                        v.tensor_scalar(
                            out=out_sb[:, p * O + h * O // 2 : p * O + (h + 1) * O // 2],
                            in0=psum[:, p * O + h * O // 2 : p * O + (h + 1) * O // 2],
                            scalar1=0.0,
                            scalar2=None,
                            op0=mybir.AluOpType.is_gt,
                        ).then_inc(sem_dve, 1)

    nc.compile()
    return nc


def _get_nc():
    if "nc" not in _NC_CACHE:
        _NC_CACHE["nc"] = _build_bass()
    return _NC_CACHE["nc"]


def _prep_inputs(input, weight_noise, bias_noise):
    import ml_dtypes

    fp8 = ml_dtypes.float8_e4m3
    x = np.asarray(input).astype(np.int8)  # (B, D, I) in {0,1}
    w = np.asarray(weight_noise).astype(np.int8)  # (D, O, I)
    bias = np.asarray(bias_noise).astype(np.float32)  # (D, O)

    xs = (2 * x - 1).astype(fp8)  # +/-1
    ws = (2 * w - 1).astype(fp8)
    # dotpm is even; the odd integer 2*floor(thr/2)+1 compares identically.
    # dotpm is in [-I, I], so clipping to +/-(I+1) changes nothing and keeps
    # every digit of the base-(256,16,1) split exactly representable in fp8.
    thr = np.float32(2.0) * bias - np.float32(I)
    thr = 2.0 * np.floor(thr.astype(np.float64) / 2.0) + 1.0
    thr = np.clip(thr, -(I + 1), I + 1)  # odd ints in [-513, 513]
    ta = np.round(thr / 128.0)
    tr = thr - 128.0 * ta
    tb = np.round(tr / 16.0)
    tc = tr - 16.0 * tb
    assert np.abs(ta).max() <= 5 and np.abs(tb).max() <= 8 and np.abs(tc).max() <= 8
    assert np.array_equal(128.0 * ta + 16.0 * tb + tc, thr)

    in_maps = []
    for cidx in range(NCORES):
        dsl = slice(cidx * DL, (cidx + 1) * DL)
        # xt[k, p, c, j, b] = xs[b, d0+2p+j, c*128+k]
        xt = xs[:, dsl, :].transpose(2, 1, 0)  # (I, DL, B)
        xt = xt.reshape(KC, 128, NP, 2, B)  # (c, k, p, j, b)
        xt = xt.transpose(1, 2, 0, 3, 4)  # (k, p, c, j, b)
        xt = np.ascontiguousarray(xt).reshape(128, XF)
        xt2 = xt
        # wt[p, k, j, c, o] = ws[d0+2p+j, o, c*128+k]
        wt = ws[dsl].transpose(0, 2, 1)  # (DL, I, O)
        wt = wt.reshape(NP, 2, KC, 128, O)  # (p, j, c, k, o)
        wt = wt.transpose(0, 3, 1, 2, 4)  # (p, k, j, c, o)
        wt = np.ascontiguousarray(wt).reshape(NP, 128, 2 * KC * O)
        # wthr[k, p, o]: rows 0:3 = (-a,-b,-c) of dir 2p, rows 4:7 of 2p+1
        th3 = np.stack([-ta[dsl], -tb[dsl], -tc[dsl], np.zeros_like(ta[dsl])])
        th3 = th3.reshape(4, NP, 2, O)  # (k, p, j, o)
        wthr = np.zeros((8, NP * O + 128), dtype=fp8)
        wthr[:, 0 : NP * O] = th3.transpose(2, 0, 1, 3).reshape(8, NP * O)
        for k, s in enumerate((128.0, 16.0, 1.0)):
            wthr[k, NP * O : NP * O + B] = s       # rows 0:3 -> even block
            wthr[4 + k, NP * O + B :] = s          # rows 4:7 -> odd block
        in_maps.append({"xt": xt2, "wt": wt, "wthr": wthr})
    return in_maps


def _patch_walrus_args():
    """Let every DGE DMA spread across all 16 DMA engines (default splits a
    DMA over ~4), so single transfers run at full aggregate bandwidth."""
    from concourse import bass_utils as bu

    if getattr(bu, "_dge_patched", False):
        return
    orig = bu.get_walrus_args

    def patched(*a, **k):
        return ["--min-num-dma-engines-for-dge=16", *orig(*a, **k)]

    bu.get_walrus_args = patched
    bu._dge_patched = True


def kernel(input, weight_noise, bias_noise):
    from concourse import bass_utils

    _patch_walrus_args()
    in_maps = _prep_inputs(input, weight_noise, bias_noise)
    nc = _get_nc()
    res = bass_utils.run_bass_kernel_spmd(nc, in_maps, core_ids=list(range(NCORES)))
    full = np.empty((B, D, O), dtype=bool)
    for cidx, r in enumerate(res.results):
        ro = np.asarray(r["out"]).reshape(128, NP, O)
        dsl = slice(cidx * DL, (cidx + 1) * DL)
        blk = full[:, dsl, :]
        blk[:, 0::2, :] = ro[0:B].astype(bool)
        blk[:, 1::2, :] = ro[B : 2 * B].astype(bool)
    return full
